# revision 3
# baseline (speedup 1.0000x reference)
"""Distributed attention kernel for 8 TRN2 NeuronCores.

Problem: B=2, T=2048, D=1024, H=16 heads, HD=64.
  q = x @ Wq.T + bq ; k = x @ Wk.T + bk ; v = q  (source quirk)
  S = q_h k_h^T / sqrt(D) ; P = softmax(S) ; o = P v_h ; concat heads.

Sharding: core c -> (batch b = c//4, head-group g = c%4, 4 heads each).
Each core is fully independent (no collectives).

v2 design notes (vs the v1 baseline at ~205us):
  - The kernel is ScalarE-bound: exp over the scores is 16.8M elements
    per core at 1 elem/cycle/lane @1.2GHz ~= 132us of ACT busy.  v1
    wasted ~35us of startup, ~32us of mid-kernel ACT idle (PE-transpose
    phases for v + head-pair boundaries) and re-throttled HAM twice.
  - v (=q in [key, dim] layout) is produced by direct PE matmuls
    x_tile.T @ Wq_cols (contracting D), not by PE transposes of qT:
    bigger ops, no transpose phase, and the psums share the "s" slots
    so no extra PSUM banks.  Bias rides in as a K=1 ones-row matmul.
    The whole v region is memset to 1.0 first so column 64 of each
    128-col slot is the ones-column folding the softmax denominator
    into PV.
  - x column-block tb0 + weights go on the sync HWDGE queues for a fast
    start (first S^T group only needs q/k cols 0:512); tb1..3 stream on
    the gpsimd SWDGE queue.  Output stores also go on gpsimd so the
    sync queues carry only the xbar transposes (avoids xbar-mode
    transitions).
  - Attention for (hp, ib=0) starts as soon as proj tb0 is done; later
    proj blocks and the second head-pair's projections fill PE slack
    under the ACT-bound attention loop.
  - Optional: a subset of exp groups is evaluated on the otherwise-idle
    VectorE via exp(e) ~= (1 + e/64)^64 (one tensor_scalar + 6 bf16
    squarings), offloading the bottleneck engine.
"""

import os
import numpy as np
import ml_dtypes

import concourse.bass as bass
import concourse.tile as tile
from concourse import bacc, mybir
from concourse.bass_utils import run_bass_kernel_spmd

B, T, D, H = 2, 2048, 1024, 16
HD = 64
NCORES = 8
HPC = 4          # heads per core
JG = HPC * HD    # 256 output dims per core
KT = 8           # contraction tiles of 128 over D
IB = 512         # query block
NIB = T // IB    # 4
NJT = T // 128   # 16 key tiles of 128
BF16 = mybir.dt.bfloat16
F32 = mybir.dt.float32

# (slot_idx 0..7, group_idx 0..10) handled by VectorE instead of ScalarE.
# slot_idx = hp*NIB + ib.
DVE_EXP_SLOTS: set = set()


def build_nc():
    nc = bacc.Bacc(None, target_bir_lowering=False, debug=False)

    xT = nc.declare_dram_parameter("xT", [1024, T], BF16, isOutput=False)
    wT = nc.declare_dram_parameter("wT", [1024, 2 * JG], BF16, isOutput=False)
    bias = nc.declare_dram_parameter("bias", [128, 4], F32, isOutput=False)
    biasrow = nc.declare_dram_parameter("biasrow", [1, JG], BF16, isOutput=False)
    out = nc.declare_dram_parameter("out", [JG + HPC, T], F32, isOutput=True)

    with tile.TileContext(nc) as tc:
        with (
            tc.tile_pool(name="const", bufs=1) as const_pool,
            tc.tile_pool(name="xw", bufs=1) as xw_pool,
            tc.tile_pool(name="qk", bufs=1) as qk_pool,
            tc.tile_pool(name="v", bufs=1) as v_pool,
            tc.tile_pool(name="p", bufs=3) as p_pool,
            tc.tile_pool(name="ev", bufs=2) as ev_pool,
            tc.tile_pool(name="psS", bufs=2, space="PSUM") as psS,
            tc.tile_pool(name="psO", bufs=2, space="PSUM") as psO,
        ):
            bias_sb = const_pool.tile([128, 4], F32, tag="bias", name="bias_sb")
            nc.sync.dma_start(bias_sb[:, :], bias[:, :])
            brow = const_pool.tile([1, JG], BF16, tag="brow", name="brow")
            nc.sync.dma_start(brow[:, :], biasrow[:, :])
            ones_t = const_pool.tile([1, 128], BF16, tag="ones", name="ones_t")
            nc.vector.memset(ones_t[:, :], 1.0)

            # v storage: 64 slots (4 heads x 16 key tiles) of 65 cols each;
            # col 64 of each slot is the ones-column (preset by the memset,
            # the transposes only overwrite cols 0..63).
            v_big = v_pool.tile([128, 128 * 4 * NJT], BF16, tag="vbig",
                                name="v_big")
            nc.vector.memset(v_big[:, :], 1.0)

            # weights + x tb0 on sync HWDGE (fast startup), x tb1..3 on the
            # gpsimd SWDGE queue.
            wt = []
            for k in range(KT):
                t_ = xw_pool.tile([128, 2 * JG], BF16, tag=f"w{k}", name=f"w{k}")
                nc.sync.dma_start(t_[:, :], wT[k * 128:(k + 1) * 128, :])
                wt.append(t_)
            xt = [xw_pool.tile([128, T], BF16, tag=f"x{k}", name=f"x{k}")
                  for k in range(KT)]
            for k in range(KT):
                nc.sync.dma_start(xt[k][:, 0:IB], xT[k * 128:(k + 1) * 128, 0:IB])
            for tb in range(1, NIB):
                cs = slice(tb * IB, (tb + 1) * IB)
                for k in range(KT):
                    nc.gpsimd.dma_start(xt[k][:, cs], xT[k * 128:(k + 1) * 128, cs])

            qT = [qk_pool.tile([128, T], BF16, tag=f"qT{j}", name=f"qT{j}")
                  for j in range(2)]
            kTt = [qk_pool.tile([128, T], BF16, tag=f"kT{j}", name=f"kT{j}")
                   for j in range(2)]

            chunks = [(jt, hh) for jt in range(NJT) for hh in range(2)]
            groups = [chunks[i:i + 3] for i in range(0, len(chunks), 3)]

            for hp in range(2):
                # ---- projections, column-block major (q then k per block);
                # v transposes issued per completed block so attention can
                # start right after tb0.
                for tb in range(NIB):
                    for w_idx, dst in ((0, qT), (1, kTt)):
                        ps = psS.tile([128, IB], F32, tag="s", name="ps_proj")
                        for k in range(KT):
                            nc.tensor.matmul(
                                ps[:, :],
                                wt[k][:, w_idx * JG + hp * 128:
                                      w_idx * JG + (hp + 1) * 128],
                                xt[k][:, tb * IB:(tb + 1) * IB],
                                start=(k == 0), stop=(k == KT - 1),
                            )
                        nc.vector.tensor_scalar(
                            dst[hp][:, tb * IB:(tb + 1) * IB], ps[:, :],
                            bias_sb[:, w_idx * 2 + hp:w_idx * 2 + hp + 1],
                            None, mybir.AluOpType.add)
                    # v natural [t, d] for the 4 key tiles this tb covers:
                    # v[t, 0:128] = sum_k x_tile[k].T @ Wq_cols[k] (+ bias row)
                    for jt in range(4 * tb, 4 * tb + 4):
                        psv = psS.tile([128, 128], F32, tag="s", name="ps_v")
                        for k in range(KT):
                            nc.tensor.matmul(
                                psv[:, :],
                                xt[k][:, jt * 128:(jt + 1) * 128],
                                wt[k][:, hp * 128:(hp + 1) * 128],
                                start=(k == 0), stop=False,
                            )
                        nc.tensor.matmul(
                            psv[:, :], ones_t[:, :],
                            brow[:, hp * 128:(hp + 1) * 128],
                            start=False, stop=True,
                        )
                        for hh in range(2):
                            vi = (hp * 2 + hh) * NJT + jt
                            nc.vector.tensor_copy(
                                v_big[:, vi * 128:vi * 128 + 64],
                                psv[:, hh * 64:(hh + 1) * 64])

                # ---- attention: S^T (2 heads row-packed) -> exp -> PV
                for ib in range(NIB):
                    slot_idx = hp * NIB + ib
                    po = [psO.tile([65, IB], F32, tag="o", name=f"po{hh}")
                          for hh in range(2)]
                    for gi, grp in enumerate(groups):
                        ng = len(grp)
                        ps = psS.tile([128, ng * IB], F32, tag="s", name="ps_s")
                        for c, (jt, hh) in enumerate(grp):
                            off = 64 * hh
                            nc.tensor.matmul(
                                ps[:, c * IB:(c + 1) * IB],
                                kTt[hp][off:off + 64, jt * 128:(jt + 1) * 128],
                                qT[hp][off:off + 64, ib * IB:(ib + 1) * IB],
                                start=True, stop=True,
                                tile_position=(off, 0),
                            )
                        pexp = p_pool.tile([128, ng * IB], BF16, tag="p",
                                           name="pexp")
                        if (slot_idx, gi) in DVE_EXP_SLOTS:
                            # exp(e) ~= (1 + e/64)^64, e = s/32
                            tmp = p_pool.tile([128, ng * IB], BF16, tag="ptmp",
                                              name="ptmp", bufs=2)
                            nc.vector.tensor_scalar(
                                pexp[:, :], ps[:, :], 1.0 / 2048.0, 1.0,
                                mybir.AluOpType.mult, mybir.AluOpType.add)
                            cur, oth = pexp, tmp
                            for _ in range(6):
                                nc.vector.tensor_tensor(
                                    oth[:, :], cur[:, :], cur[:, :],
                                    mybir.AluOpType.mult)
                                cur, oth = oth, cur
                            # 6 squarings end back in pexp
                        else:
                            nc.scalar.activation(
                                pexp[:, :], ps[:, :],
                                mybir.ActivationFunctionType.Exp,
                                scale=1.0 / 32.0,
                            )
                        for c, (jt, hh) in enumerate(grp):
                            vi = (hp * 2 + hh) * NJT + jt
                            nc.tensor.matmul(
                                po[hh][:, :],
                                v_big[:, vi * 128:vi * 128 + 65],
                                pexp[:, c * IB:(c + 1) * IB],
                                start=(jt == 0), stop=(jt == NJT - 1),
                            )
                    for hh in range(2):
                        h = 2 * hp + hh
                        ev = ev_pool.tile([65, IB], F32, tag="ev", name="ev")
                        nc.vector.tensor_copy(ev[:, :], po[hh][:, :])
                        nc.gpsimd.dma_start(
                            out[h * HD:(h + 1) * HD, ib * IB:(ib + 1) * IB],
                            ev[0:64, :])
                        nc.gpsimd.dma_start(
                            out[JG + h:JG + h + 1, ib * IB:(ib + 1) * IB],
                            ev[64:65, :])
    nc.finalize()
    return nc


_NC_CACHE = None


def _ensure_ntff_hook():
    """Provide the antenv.axon_hooks NTFF-profiling shim this image lacks."""
    import sys
    import types
    import ctypes
    import contextlib

    if "antenv.axon_hooks" in sys.modules:
        return
    mod = types.ModuleType("antenv.axon_hooks")
    state = {"hook": None}
    mod.set_axon_ntff_profile_hook = lambda h: state.__setitem__("hook", h)
    mod.get_axon_ntff_profile_hook = lambda: state["hook"]
    sys.modules["antenv.axon_hooks"] = mod
    try:
        import antenv
        antenv.axon_hooks = mod
    except ImportError:
        pass
    so = "/opt/axon/libaxon_pjrt.so"
    if not os.path.exists(so):
        return
    lib = ctypes.CDLL(so)
    if not hasattr(lib, "axon_start_nrt_profile"):
        return
    lib.axon_start_nrt_profile.argtypes = [
        ctypes.POINTER(ctypes.c_int64), ctypes.c_size_t]
    lib.axon_start_nrt_profile.restype = ctypes.c_int64
    lib.axon_stop_nrt_profile.argtypes = [ctypes.c_char_p]
    lib.axon_stop_nrt_profile.restype = ctypes.c_int64

    @contextlib.contextmanager
    def _hook(output_dir, device_ids):
        import jax
        jax.devices()
        if device_ids:
            ids = (ctypes.c_int64 * len(device_ids))(*device_ids)
            rc = lib.axon_start_nrt_profile(ids, len(device_ids))
        else:
            rc = lib.axon_start_nrt_profile(None, 0)
        if rc != 0:
            raise RuntimeError(f"axon_start_nrt_profile rc={rc}")
        try:
            yield
        finally:
            n = lib.axon_stop_nrt_profile(str(output_dir).encode())
            print(f"ntff profile: {n} file(s) -> {output_dir}")

    mod.set_axon_ntff_profile_hook(_hook)


def kernel(x, Wq, bq, Wk, bk):
    global _NC_CACHE
    x = np.asarray(x, dtype=np.float32)
    Wq = np.asarray(Wq, dtype=np.float32)
    bq = np.asarray(bq, dtype=np.float32)
    Wk = np.asarray(Wk, dtype=np.float32)
    bk = np.asarray(bk, dtype=np.float32)

    bf = ml_dtypes.bfloat16
    in_maps = []
    for c in range(NCORES):
        b, g = c // 4, c % 4
        sl = slice(g * JG, (g + 1) * JG)
        w_all = np.concatenate([Wq[sl].T, Wk[sl].T], axis=1)  # [1024, 512]
        bias_all = np.stack(
            [bq[sl][0:128], bq[sl][128:256],
             bk[sl][0:128], bk[sl][128:256]], axis=1)  # [128, 4]
        in_maps.append({
            "xT": np.ascontiguousarray(x[b].T).astype(bf),
            "wT": w_all.astype(bf),
            "bias": bias_all.astype(np.float32),
            "biasrow": bq[sl].reshape(1, JG).astype(bf),
        })

    if _NC_CACHE is None:
        _NC_CACHE = build_nc()
    nc = _NC_CACHE

    if int(os.environ.get("KERNEL_TRACE", "0")):
        _ensure_ntff_hook()
    res = run_bass_kernel_spmd(
        nc, in_maps, core_ids=list(range(NCORES)),
        trace=bool(int(os.environ.get("KERNEL_TRACE", "0"))),
        tmpdir=os.environ.get("KERNEL_TMPDIR") or None,
    )
    if res.exec_time_ns is not None:
        print(f"HW exec time: {res.exec_time_ns} ns")

    full = np.empty((B, T, D), np.float32)
    for c in range(NCORES):
        b, g = c // 4, c % 4
        oc = res.results[c]["out"]            # [260, 2048] f32
        o = oc[0:JG].reshape(HPC, HD, T)      # [4, 64, 2048]
        s = oc[JG:JG + HPC].reshape(HPC, 1, T)
        blk = (o / s).transpose(2, 0, 1).reshape(T, JG)
        full[b, :, g * JG:(g + 1) * JG] = blk
    return full


# revision 4
# speedup vs baseline: 1.1314x; 1.1314x over previous
"""Distributed attention kernel for 8 TRN2 NeuronCores.

Problem: B=2, T=2048, D=1024, H=16 heads, HD=64.
  q = x @ Wq.T + bq ; k = x @ Wk.T + bk ; v = q  (source quirk)
  S = q_h k_h^T / sqrt(D) ; P = softmax(S) ; o = P v_h ; concat heads.

Sharding: core c -> (batch b = c//4, head-group g = c%4, 4 heads each).
Each core is fully independent (no collectives).

v4 design notes (vs the v1 baseline at ~205us):
  - The kernel is ScalarE-bound: exp over the scores is 16.8M elements
    per core at 1 elem/cycle/lane @1.2GHz ~= 132us of ACT busy.  v1
    wasted ~35us of startup, ~32us of mid-kernel ACT idle (PE-transpose
    phases for v + head-pair boundaries) and re-throttled HAM twice.
  - PSUM layout is the key constraint: S^T exp groups are one jt-pair
    (FD=1024, both heads side by side -> every S^T matmul runs as a
    row-packed concurrent pair), double-buffered = 4 banks; po
    accumulators = 2 banks; the remaining 2 banks are a dedicated
    prework tag ("w") so projections and v-transposes overlap the
    attention pipeline without stealing its psum slots (that slot
    contention is what serialized v1/v3).
  - v (=q in [key, dim] layout): one [128,128] PE transpose per
    (head-pair, key tile) gives both heads' v tiles at once; a single
    strided DVE copy evicts both.  The v region is memset to 1.0 so
    col 64 of each 128-col slot is the ones-column that folds the
    softmax denominator into PV.
  - x column-block tb0 + weights go on sync HWDGE queues (fast start);
    tb1..3 stream on the gpsimd SWDGE queue; output stores also on
    gpsimd.
  - Issue order: per head-pair, prework first then attention; the
    2-slot "w" rotation rations prework so the scheduler interleaves
    it into PE gaps under the ACT-bound attention loop, and hp1's
    prework (lower priority) fills hp0's attention slack.
  - Optional: DVE_EXP_SLOTS marks exp groups evaluated on the
    otherwise-idle VectorE via exp(e) ~= (1 + e/64)^64 (tensor_scalar
    + 6 bf16 squarings), offloading the bottleneck engine.
"""

import os
import numpy as np
import ml_dtypes

import concourse.bass as bass
import concourse.tile as tile
from concourse import bacc, mybir
from concourse.bass_utils import run_bass_kernel_spmd

B, T, D, H = 2, 2048, 1024, 16
HD = 64
NCORES = 8
HPC = 4          # heads per core
JG = HPC * HD    # 256 output dims per core
KT = 8           # contraction tiles of 128 over D
IB = 512         # query block
NIB = T // IB    # 4
NJT = T // 128   # 16 key tiles of 128
BF16 = mybir.dt.bfloat16
F32 = mybir.dt.float32

# (slot_idx 0..7, jt 0..15) handled by VectorE instead of ScalarE.
# slot_idx = hp*NIB + ib.
DVE_EXP_SLOTS: set = set()


def build_nc():
    nc = bacc.Bacc(None, target_bir_lowering=False, debug=False)

    xT = nc.declare_dram_parameter("xT", [1024, T], BF16, isOutput=False)
    wT = nc.declare_dram_parameter("wT", [1024, 2 * JG], BF16, isOutput=False)
    bias = nc.declare_dram_parameter("bias", [128, 4], F32, isOutput=False)
    idn = nc.declare_dram_parameter("idn", [128, 128], BF16, isOutput=False)
    out = nc.declare_dram_parameter("out", [JG + HPC, T], F32, isOutput=True)

    with tile.TileContext(nc) as tc:
        with (
            tc.tile_pool(name="const", bufs=1) as const_pool,
            tc.tile_pool(name="xw", bufs=1) as xw_pool,
            tc.tile_pool(name="qk", bufs=1) as qk_pool,
            tc.tile_pool(name="v", bufs=1) as v_pool,
            tc.tile_pool(name="p", bufs=3) as p_pool,
            tc.tile_pool(name="ev", bufs=2) as ev_pool,
            tc.tile_pool(name="psS", bufs=2, space="PSUM") as psS,
            tc.tile_pool(name="psW", bufs=2, space="PSUM") as psW,
            tc.tile_pool(name="psO", bufs=2, space="PSUM") as psO,
        ):
            bias_sb = const_pool.tile([128, 4], F32, tag="bias", name="bias_sb")
            nc.sync.dma_start(bias_sb[:, :], bias[:, :])
            ident = const_pool.tile([128, 128], BF16, tag="ident", name="ident")
            nc.sync.dma_start(ident[:, :], idn[:, :])

            # v storage: 64 slots (4 heads x 16 key tiles) of 128 cols each;
            # cols 0..63 = v data, col 64 = ones (preset by the memset).
            v_big = v_pool.tile([128, 128 * 4 * NJT], BF16, tag="vbig",
                                name="v_big")
            nc.vector.memset(v_big[:, :], 1.0)

            # weights + x tb0 on sync HWDGE (fast startup), x tb1..3 on the
            # gpsimd SWDGE queue.
            wt = []
            for k in range(KT):
                t_ = xw_pool.tile([128, 2 * JG], BF16, tag=f"w{k}", name=f"w{k}")
                nc.sync.dma_start(t_[:, :], wT[k * 128:(k + 1) * 128, :])
                wt.append(t_)
            xt = [xw_pool.tile([128, T], BF16, tag=f"x{k}", name=f"x{k}")
                  for k in range(KT)]
            for k in range(KT):
                nc.sync.dma_start(xt[k][:, 0:IB], xT[k * 128:(k + 1) * 128, 0:IB])
            for tb in range(1, NIB):
                cs = slice(tb * IB, (tb + 1) * IB)
                for k in range(KT):
                    nc.gpsimd.dma_start(xt[k][:, cs], xT[k * 128:(k + 1) * 128, cs])

            qT = [qk_pool.tile([128, T], BF16, tag=f"qT{j}", name=f"qT{j}")
                  for j in range(2)]
            kTt = [qk_pool.tile([128, T], BF16, tag=f"kT{j}", name=f"kT{j}")
                   for j in range(2)]

            for hp in range(2):
                # ---- prework: projections (column-block major, q then k)
                # and the paired v transposes, all on the dedicated "w"
                # psum slots so they never stall the attention pipeline.
                for tb in range(NIB):
                    for w_idx, dst in ((0, qT), (1, kTt)):
                        ps = psW.tile([128, IB], F32, tag="w", name="ps_proj")
                        for k in range(KT):
                            nc.tensor.matmul(
                                ps[:, :],
                                wt[k][:, w_idx * JG + hp * 128:
                                      w_idx * JG + (hp + 1) * 128],
                                xt[k][:, tb * IB:(tb + 1) * IB],
                                start=(k == 0), stop=(k == KT - 1),
                            )
                        nc.vector.tensor_scalar(
                            dst[hp][:, tb * IB:(tb + 1) * IB], ps[:, :],
                            bias_sb[:, w_idx * 2 + hp:w_idx * 2 + hp + 1],
                            None, mybir.AluOpType.add)
                    # v for the 4 key tiles this tb covers: one [128,128]
                    # PE transpose gives both heads' v tiles; one strided
                    # DVE copy evicts both (slot stride 128 keeps the xbar
                    # -free layout and the preset ones columns).
                    for jt in range(4 * tb, 4 * tb + 4):
                        pt = psW.tile([128, 128], BF16, tag="w", name="pt_tr")
                        nc.tensor.transpose(
                            pt[:, :], qT[hp][:, jt * 128:(jt + 1) * 128],
                            ident[:, :])
                        for hh in range(2):
                            vi = (hp * 2 + hh) * NJT + jt
                            nc.vector.tensor_copy(
                                v_big[:, vi * 128:vi * 128 + 64],
                                pt[:, hh * 64:(hh + 1) * 64])

                # ---- attention: per query block, 16 jt-pair groups of
                # S^T (row-packed pair) -> exp -> PV.
                for ib in range(NIB):
                    slot_idx = hp * NIB + ib
                    po = [psO.tile([65, IB], F32, tag="o", name=f"po{hh}")
                          for hh in range(2)]
                    for jt in range(NJT):
                        ps = psS.tile([128, 2 * IB], F32, tag="s", name="ps_s")
                        for hh in range(2):
                            off = 64 * hh
                            nc.tensor.matmul(
                                ps[:, hh * IB:(hh + 1) * IB],
                                kTt[hp][off:off + 64, jt * 128:(jt + 1) * 128],
                                qT[hp][off:off + 64, ib * IB:(ib + 1) * IB],
                                start=True, stop=True,
                                tile_position=(off, 0),
                            )
                        pexp = p_pool.tile([128, 2 * IB], BF16, tag="p",
                                           name="pexp")
                        if (slot_idx, jt) in DVE_EXP_SLOTS:
                            # exp(e) ~= (1 + e/64)^64, e = s/32
                            tmp = p_pool.tile([128, 2 * IB], BF16, tag="ptmp",
                                              name="ptmp", bufs=2)
                            nc.vector.tensor_scalar(
                                pexp[:, :], ps[:, :], 1.0 / 2048.0, 1.0,
                                mybir.AluOpType.mult, mybir.AluOpType.add)
                            cur, oth = pexp, tmp
                            for _ in range(6):
                                nc.vector.tensor_tensor(
                                    oth[:, :], cur[:, :], cur[:, :],
                                    mybir.AluOpType.mult)
                                cur, oth = oth, cur
                            # 6 squarings end back in pexp
                        else:
                            nc.scalar.activation(
                                pexp[:, :], ps[:, :],
                                mybir.ActivationFunctionType.Exp,
                                scale=1.0 / 32.0,
                            )
                        for hh in range(2):
                            vi = (hp * 2 + hh) * NJT + jt
                            nc.tensor.matmul(
                                po[hh][:, :],
                                v_big[:, vi * 128:vi * 128 + 65],
                                pexp[:, hh * IB:(hh + 1) * IB],
                                start=(jt == 0), stop=(jt == NJT - 1),
                            )
                    for hh in range(2):
                        h = 2 * hp + hh
                        ev = ev_pool.tile([65, IB], F32, tag="ev", name="ev")
                        nc.vector.tensor_copy(ev[:, :], po[hh][:, :])
                        nc.gpsimd.dma_start(
                            out[h * HD:(h + 1) * HD, ib * IB:(ib + 1) * IB],
                            ev[0:64, :])
                        nc.gpsimd.dma_start(
                            out[JG + h:JG + h + 1, ib * IB:(ib + 1) * IB],
                            ev[64:65, :])
    nc.finalize()
    return nc


_NC_CACHE = None


def _ensure_ntff_hook():
    """Provide the antenv.axon_hooks NTFF-profiling shim this image lacks."""
    import sys
    import types
    import ctypes
    import contextlib

    if "antenv.axon_hooks" in sys.modules:
        return
    mod = types.ModuleType("antenv.axon_hooks")
    state = {"hook": None}
    mod.set_axon_ntff_profile_hook = lambda h: state.__setitem__("hook", h)
    mod.get_axon_ntff_profile_hook = lambda: state["hook"]
    sys.modules["antenv.axon_hooks"] = mod
    try:
        import antenv
        antenv.axon_hooks = mod
    except ImportError:
        pass
    so = "/opt/axon/libaxon_pjrt.so"
    if not os.path.exists(so):
        return
    lib = ctypes.CDLL(so)
    if not hasattr(lib, "axon_start_nrt_profile"):
        return
    lib.axon_start_nrt_profile.argtypes = [
        ctypes.POINTER(ctypes.c_int64), ctypes.c_size_t]
    lib.axon_start_nrt_profile.restype = ctypes.c_int64
    lib.axon_stop_nrt_profile.argtypes = [ctypes.c_char_p]
    lib.axon_stop_nrt_profile.restype = ctypes.c_int64

    @contextlib.contextmanager
    def _hook(output_dir, device_ids):
        import jax
        jax.devices()
        if device_ids:
            ids = (ctypes.c_int64 * len(device_ids))(*device_ids)
            rc = lib.axon_start_nrt_profile(ids, len(device_ids))
        else:
            rc = lib.axon_start_nrt_profile(None, 0)
        if rc != 0:
            raise RuntimeError(f"axon_start_nrt_profile rc={rc}")
        try:
            yield
        finally:
            n = lib.axon_stop_nrt_profile(str(output_dir).encode())
            print(f"ntff profile: {n} file(s) -> {output_dir}")

    mod.set_axon_ntff_profile_hook(_hook)


def kernel(x, Wq, bq, Wk, bk):
    global _NC_CACHE
    x = np.asarray(x, dtype=np.float32)
    Wq = np.asarray(Wq, dtype=np.float32)
    bq = np.asarray(bq, dtype=np.float32)
    Wk = np.asarray(Wk, dtype=np.float32)
    bk = np.asarray(bk, dtype=np.float32)

    bf = ml_dtypes.bfloat16
    in_maps = []
    for c in range(NCORES):
        b, g = c // 4, c % 4
        sl = slice(g * JG, (g + 1) * JG)
        w_all = np.concatenate([Wq[sl].T, Wk[sl].T], axis=1)  # [1024, 512]
        bias_all = np.stack(
            [bq[sl][0:128], bq[sl][128:256],
             bk[sl][0:128], bk[sl][128:256]], axis=1)  # [128, 4]
        in_maps.append({
            "xT": np.ascontiguousarray(x[b].T).astype(bf),
            "wT": w_all.astype(bf),
            "bias": bias_all.astype(np.float32),
            "idn": np.eye(128, dtype=np.float32).astype(bf),
        })

    if _NC_CACHE is None:
        _NC_CACHE = build_nc()
    nc = _NC_CACHE

    if int(os.environ.get("KERNEL_TRACE", "0")):
        _ensure_ntff_hook()
    res = run_bass_kernel_spmd(
        nc, in_maps, core_ids=list(range(NCORES)),
        trace=bool(int(os.environ.get("KERNEL_TRACE", "0"))),
        tmpdir=os.environ.get("KERNEL_TMPDIR") or None,
    )
    if res.exec_time_ns is not None:
        print(f"HW exec time: {res.exec_time_ns} ns")

    full = np.empty((B, T, D), np.float32)
    for c in range(NCORES):
        b, g = c // 4, c % 4
        oc = res.results[c]["out"]            # [260, 2048] f32
        o = oc[0:JG].reshape(HPC, HD, T)      # [4, 64, 2048]
        s = oc[JG:JG + HPC].reshape(HPC, 1, T)
        blk = (o / s).transpose(2, 0, 1).reshape(T, JG)
        full[b, :, g * JG:(g + 1) * JG] = blk
    return full


# revision 5
# speedup vs baseline: 1.1840x; 1.0465x over previous
"""Distributed attention kernel for 8 TRN2 NeuronCores.

Problem: B=2, T=2048, D=1024, H=16 heads, HD=64.
  q = x @ Wq.T + bq ; k = x @ Wk.T + bk ; v = q  (source quirk)
  S = q_h k_h^T / sqrt(D) ; P = softmax(S) ; o = P v_h ; concat heads.

Sharding: core c -> (batch b = c//4, head-group g = c%4, 4 heads each).
Each core is fully independent (no collectives).

v4 design notes (vs the v1 baseline at ~205us):
  - The kernel is ScalarE-bound: exp over the scores is 16.8M elements
    per core at 1 elem/cycle/lane @1.2GHz ~= 132us of ACT busy.  v1
    wasted ~35us of startup, ~32us of mid-kernel ACT idle (PE-transpose
    phases for v + head-pair boundaries) and re-throttled HAM twice.
  - PSUM layout is the key constraint: S^T exp groups are one jt-pair
    (FD=1024, both heads side by side -> every S^T matmul runs as a
    row-packed concurrent pair), double-buffered = 4 banks; po
    accumulators = 2 banks; the remaining 2 banks are a dedicated
    prework tag ("w") so projections and v-transposes overlap the
    attention pipeline without stealing its psum slots (that slot
    contention is what serialized v1/v3).
  - v (=q in [key, dim] layout): one [128,128] PE transpose per
    (head-pair, key tile) gives both heads' v tiles at once; a single
    strided DVE copy evicts both.  The v region is memset to 1.0 so
    col 64 of each 128-col slot is the ones-column that folds the
    softmax denominator into PV.
  - x column-block tb0 + weights go on sync HWDGE queues (fast start);
    tb1..3 stream on the gpsimd SWDGE queue; output stores also on
    gpsimd.
  - Issue order: per head-pair, prework first then attention; the
    2-slot "w" rotation rations prework so the scheduler interleaves
    it into PE gaps under the ACT-bound attention loop, and hp1's
    prework (lower priority) fills hp0's attention slack.
  - Optional: DVE_EXP_SLOTS marks exp groups evaluated on the
    otherwise-idle VectorE via exp(e) ~= (1 + e/64)^64 (tensor_scalar
    + 6 bf16 squarings), offloading the bottleneck engine.
"""

import os
import numpy as np
import ml_dtypes

import concourse.bass as bass
import concourse.tile as tile
from concourse import bacc, mybir
from concourse.bass_utils import run_bass_kernel_spmd

B, T, D, H = 2, 2048, 1024, 16
HD = 64
NCORES = 8
HPC = 4          # heads per core
JG = HPC * HD    # 256 output dims per core
KT = 8           # contraction tiles of 128 over D
IB = 512         # query block
NIB = T // IB    # 4
NJT = T // 128   # 16 key tiles of 128
BF16 = mybir.dt.bfloat16
F32 = mybir.dt.float32

# (slot_idx 0..7, jt 0..15) handled by VectorE instead of ScalarE.
# slot_idx = hp*NIB + ib.
DVE_EXP_SLOTS: set = set()


def build_nc():
    nc = bacc.Bacc(None, target_bir_lowering=False, debug=False)

    xT = nc.declare_dram_parameter("xT", [1024, T], BF16, isOutput=False)
    wT = nc.declare_dram_parameter("wT", [1024, 2 * JG], BF16, isOutput=False)
    bias = nc.declare_dram_parameter("bias", [128, 4], F32, isOutput=False)
    idn = nc.declare_dram_parameter("idn", [128, 128], BF16, isOutput=False)
    out = nc.declare_dram_parameter("out", [JG + HPC, T], F32, isOutput=True)

    with tile.TileContext(nc) as tc:
        with (
            tc.tile_pool(name="const", bufs=1) as const_pool,
            tc.tile_pool(name="xw", bufs=1) as xw_pool,
            tc.tile_pool(name="qk", bufs=1) as qk_pool,
            tc.tile_pool(name="v", bufs=1) as v_pool,
            tc.tile_pool(name="p", bufs=3) as p_pool,
            tc.tile_pool(name="ev", bufs=2) as ev_pool,
            tc.tile_pool(name="psS", bufs=2, space="PSUM") as psS,
            tc.tile_pool(name="psW", bufs=2, space="PSUM") as psW,
            tc.tile_pool(name="psO", bufs=2, space="PSUM") as psO,
        ):
            # v storage: 64 slots (4 heads x 16 key tiles) of 128 cols each;
            # cols 0..63 = v data, col 64 = ones (preset by the memset).
            v_big = v_pool.tile([128, 128 * 4 * NJT], BF16, tag="vbig",
                                name="v_big")
            nc.vector.memset(v_big[:, :], 1.0)

            # Startup-critical loads (first proj chain needs all 8 w tiles +
            # all 8 x tb0 tiles): interleave across both DMA queues first;
            # ident/bias follow; x tb1..3 stream behind.
            wt = [xw_pool.tile([128, 2 * JG], BF16, tag=f"w{k}", name=f"w{k}")
                  for k in range(KT)]
            xt = [xw_pool.tile([128, T], BF16, tag=f"x{k}", name=f"x{k}")
                  for k in range(KT)]
            for k in range(KT):
                eng = nc.sync if k % 2 == 0 else nc.gpsimd
                eng.dma_start(wt[k][:, :], wT[k * 128:(k + 1) * 128, :])
            for k in range(KT):
                eng = nc.sync if k % 2 == 0 else nc.gpsimd
                eng.dma_start(xt[k][:, 0:IB], xT[k * 128:(k + 1) * 128, 0:IB])
            ident = const_pool.tile([128, 128], BF16, tag="ident", name="ident")
            nc.sync.dma_start(ident[:, :], idn[:, :])
            bias_sb = const_pool.tile([128, 4], F32, tag="bias", name="bias_sb")
            nc.sync.dma_start(bias_sb[:, :], bias[:, :])
            for tb in range(1, NIB):
                cs = slice(tb * IB, (tb + 1) * IB)
                for k in range(KT):
                    eng = nc.sync if k % 2 == 0 else nc.gpsimd
                    eng.dma_start(xt[k][:, cs], xT[k * 128:(k + 1) * 128, cs])

            qT = [qk_pool.tile([128, T], BF16, tag=f"qT{j}", name=f"qT{j}")
                  for j in range(2)]
            kTt = [qk_pool.tile([128, T], BF16, tag=f"kT{j}", name=f"kT{j}")
                   for j in range(2)]

            # ---- prework units (run on the dedicated "w" psum slots) ----
            def proj_unit(hp, tb, w_idx):
                dst = qT if w_idx == 0 else kTt
                ps = psW.tile([128, IB], F32, tag="w", name="ps_proj")
                for k in range(KT):
                    nc.tensor.matmul(
                        ps[:, :],
                        wt[k][:, w_idx * JG + hp * 128:
                              w_idx * JG + (hp + 1) * 128],
                        xt[k][:, tb * IB:(tb + 1) * IB],
                        start=(k == 0), stop=(k == KT - 1),
                    )
                nc.vector.tensor_scalar(
                    dst[hp][:, tb * IB:(tb + 1) * IB], ps[:, :],
                    bias_sb[:, w_idx * 2 + hp:w_idx * 2 + hp + 1],
                    None, mybir.AluOpType.add)

            def trans_unit(hp, jt):
                # one [128,128] PE transpose yields both heads' v tiles
                pt = psW.tile([128, 128], BF16, tag="w", name="pt_tr")
                nc.tensor.transpose(
                    pt[:, :], qT[hp][:, jt * 128:(jt + 1) * 128], ident[:, :])
                for hh in range(2):
                    vi = (hp * 2 + hh) * NJT + jt
                    nc.vector.tensor_copy(
                        v_big[:, vi * 128:vi * 128 + 64],
                        pt[:, hh * 64:(hh + 1) * 64])

            def unit_list(hp):
                us = []
                for tb in range(NIB):
                    us.append(lambda hp=hp, tb=tb: proj_unit(hp, tb, 0))
                    us.append(lambda hp=hp, tb=tb: proj_unit(hp, tb, 1))
                    for jt in range(4 * tb, 4 * tb + 4):
                        us.append(lambda hp=hp, jt=jt: trans_unit(hp, jt))
                return us

            # ---- one attention group: S^T jt-pair -> exp -> PV pair ----
            def attn_group(hp, ib, jt, po):
                slot_idx = hp * NIB + ib
                ps = psS.tile([128, 2 * IB], F32, tag="s", name="ps_s")
                for hh in range(2):
                    off = 64 * hh
                    nc.tensor.matmul(
                        ps[:, hh * IB:(hh + 1) * IB],
                        kTt[hp][off:off + 64, jt * 128:(jt + 1) * 128],
                        qT[hp][off:off + 64, ib * IB:(ib + 1) * IB],
                        start=True, stop=True,
                        tile_position=(off, 0),
                    )
                pexp = p_pool.tile([128, 2 * IB], BF16, tag="p", name="pexp")
                if (slot_idx, jt) in DVE_EXP_SLOTS:
                    # exp(e) ~= (1 + e/64)^64, e = s/32
                    tmp = p_pool.tile([128, 2 * IB], BF16, tag="ptmp",
                                      name="ptmp", bufs=2)
                    nc.vector.tensor_scalar(
                        pexp[:, :], ps[:, :], 1.0 / 2048.0, 1.0,
                        mybir.AluOpType.mult, mybir.AluOpType.add)
                    cur, oth = pexp, tmp
                    for _ in range(6):
                        nc.vector.tensor_tensor(
                            oth[:, :], cur[:, :], cur[:, :],
                            mybir.AluOpType.mult)
                        cur, oth = oth, cur
                    # 6 squarings end back in pexp
                else:
                    nc.scalar.activation(
                        pexp[:, :], ps[:, :],
                        mybir.ActivationFunctionType.Exp,
                        scale=1.0 / 32.0,
                    )
                for hh in range(2):
                    vi = (hp * 2 + hh) * NJT + jt
                    nc.tensor.matmul(
                        po[hh][:, :],
                        v_big[:, vi * 128:vi * 128 + 65],
                        pexp[:, hh * IB:(hh + 1) * IB],
                        start=(jt == 0), stop=(jt == NJT - 1),
                    )

            def evict(hp, ib, po):
                for hh in range(2):
                    h = 2 * hp + hh
                    ev = ev_pool.tile([65, IB], F32, tag="ev", name="ev")
                    nc.vector.tensor_copy(ev[:, :], po[hh][:, :])
                    nc.gpsimd.dma_start(
                        out[h * HD:(h + 1) * HD, ib * IB:(ib + 1) * IB],
                        ev[0:64, :])
                    nc.gpsimd.dma_start(
                        out[JG + h:JG + h + 1, ib * IB:(ib + 1) * IB],
                        ev[64:65, :])

            # ---- issue order: drip prework between attention groups so
            # the scheduler's issue-order priority alternates.  hp0's
            # remaining units drip 2 per ib0-group; hp1's units drip 1 per
            # group across hp0's ib1..3; hp1's attention then runs clean.
            units0, units1 = unit_list(0), unit_list(1)
            for u in units0[:4]:
                u()
            q0 = units0[4:]
            q1 = list(units1)
            for hp in range(2):
                for ib in range(NIB):
                    po = [psO.tile([65, IB], F32, tag="o", name=f"po{hh}")
                          for hh in range(2)]
                    for jt in range(NJT):
                        if hp == 0 and ib == 0:
                            for u in q0[:2]:
                                u()
                            del q0[:2]
                        elif hp == 0 and jt % 2 == 0 and q1:
                            q1.pop(0)()
                        attn_group(hp, ib, jt, po)
                    evict(hp, ib, po)
    nc.finalize()
    return nc


_NC_CACHE = None


def _ensure_ntff_hook():
    """Provide the antenv.axon_hooks NTFF-profiling shim this image lacks."""
    import sys
    import types
    import ctypes
    import contextlib

    if "antenv.axon_hooks" in sys.modules:
        return
    mod = types.ModuleType("antenv.axon_hooks")
    state = {"hook": None}
    mod.set_axon_ntff_profile_hook = lambda h: state.__setitem__("hook", h)
    mod.get_axon_ntff_profile_hook = lambda: state["hook"]
    sys.modules["antenv.axon_hooks"] = mod
    try:
        import antenv
        antenv.axon_hooks = mod
    except ImportError:
        pass
    so = "/opt/axon/libaxon_pjrt.so"
    if not os.path.exists(so):
        return
    lib = ctypes.CDLL(so)
    if not hasattr(lib, "axon_start_nrt_profile"):
        return
    lib.axon_start_nrt_profile.argtypes = [
        ctypes.POINTER(ctypes.c_int64), ctypes.c_size_t]
    lib.axon_start_nrt_profile.restype = ctypes.c_int64
    lib.axon_stop_nrt_profile.argtypes = [ctypes.c_char_p]
    lib.axon_stop_nrt_profile.restype = ctypes.c_int64

    @contextlib.contextmanager
    def _hook(output_dir, device_ids):
        import jax
        jax.devices()
        if device_ids:
            ids = (ctypes.c_int64 * len(device_ids))(*device_ids)
            rc = lib.axon_start_nrt_profile(ids, len(device_ids))
        else:
            rc = lib.axon_start_nrt_profile(None, 0)
        if rc != 0:
            raise RuntimeError(f"axon_start_nrt_profile rc={rc}")
        try:
            yield
        finally:
            n = lib.axon_stop_nrt_profile(str(output_dir).encode())
            print(f"ntff profile: {n} file(s) -> {output_dir}")

    mod.set_axon_ntff_profile_hook(_hook)


def kernel(x, Wq, bq, Wk, bk):
    global _NC_CACHE
    x = np.asarray(x, dtype=np.float32)
    Wq = np.asarray(Wq, dtype=np.float32)
    bq = np.asarray(bq, dtype=np.float32)
    Wk = np.asarray(Wk, dtype=np.float32)
    bk = np.asarray(bk, dtype=np.float32)

    bf = ml_dtypes.bfloat16
    in_maps = []
    for c in range(NCORES):
        b, g = c // 4, c % 4
        sl = slice(g * JG, (g + 1) * JG)
        w_all = np.concatenate([Wq[sl].T, Wk[sl].T], axis=1)  # [1024, 512]
        bias_all = np.stack(
            [bq[sl][0:128], bq[sl][128:256],
             bk[sl][0:128], bk[sl][128:256]], axis=1)  # [128, 4]
        in_maps.append({
            "xT": np.ascontiguousarray(x[b].T).astype(bf),
            "wT": w_all.astype(bf),
            "bias": bias_all.astype(np.float32),
            "idn": np.eye(128, dtype=np.float32).astype(bf),
        })

    if _NC_CACHE is None:
        _NC_CACHE = build_nc()
    nc = _NC_CACHE

    if int(os.environ.get("KERNEL_TRACE", "0")):
        _ensure_ntff_hook()
    res = run_bass_kernel_spmd(
        nc, in_maps, core_ids=list(range(NCORES)),
        trace=bool(int(os.environ.get("KERNEL_TRACE", "0"))),
        tmpdir=os.environ.get("KERNEL_TMPDIR") or None,
    )
    if res.exec_time_ns is not None:
        print(f"HW exec time: {res.exec_time_ns} ns")

    full = np.empty((B, T, D), np.float32)
    for c in range(NCORES):
        b, g = c // 4, c % 4
        oc = res.results[c]["out"]            # [260, 2048] f32
        o = oc[0:JG].reshape(HPC, HD, T)      # [4, 64, 2048]
        s = oc[JG:JG + HPC].reshape(HPC, 1, T)
        blk = (o / s).transpose(2, 0, 1).reshape(T, JG)
        full[b, :, g * JG:(g + 1) * JG] = blk
    return full
